# revision 13
# baseline (speedup 1.0000x reference)
"""BPMLL loss kernel for Trainium2, data-parallel over 8 NeuronCores.

Reference math (B=512 rows, n=512 labels, bias=(1,1)):
    s   = sigmoid(c)
    pos = sum_k y_k * exp(-s_k)         (per row)
    neg = sum_l (1-y_l) * exp(+s_l)     (per row)
    loss = mean( pos*neg / (|Y| * |Ybar|) )

Device formulation (avoids the sigmoid/exp table-set switch on the ACT
engine -- tanh and exp live in the same `exp_and_others` table set):
    v  = 1 - 2y  in {+1,-1}
    exp(v*s) = exp(v*(0.5 + 0.5*tanh(c/2))) = e^{v/2} * exp(0.5*tanh(v*c/2))
    A  = exp(0.5*tanh(v*c/2))            (tanh odd => tanh(v*c/2)=v*tanh(c/2))
    SA = sum_k A_k,  SW2 = sum_k (y_k-0.5)*A_k = -SW/2,  SVr = sum_k -2*y_k
    pos*neg       = (SA^2 - SW^2)/4      (the e^{+-1/2} factors cancel)
    |Y|*|Ybar|    = (n^2 - SV^2)/4,  SV = n + SVr
    loss_row      = (SA^2 - 4*SW2^2) / -((SVr + 2n)*SVr)

Each core processes 64 rows laid out as [128 partitions, 256 free]:
partition p<64 holds row p cols 0:256, partition p+64 holds row p cols
256:512. Row sums are recovered by folding partitions p and p+64.

The 64 per-row losses are gathered into one partition with a 32x32
stream transpose before the output DMA so the store is a single
contiguous 256B descriptor (a [64,1] per-partition store pays ~370ns
per DMA-semaphore increment, ~6us total).
"""

import numpy as np

import concourse.bacc as bacc
import concourse.mybir as mybir
import concourse.tile as tile
from concourse.bass_utils import run_bass_kernel_spmd

N_CORES = 8
B = 512          # batch rows
N = 512          # labels per row
RB = B // N_CORES  # 64 rows per core
H = 2              # column halves folded onto partitions
P = RB * H         # 128 partitions
FD = N // H        # 256 free elements per partition

_NC_CACHE = {}


def build_nc():
    AF = mybir.ActivationFunctionType
    OP = mybir.AluOpType
    f32 = mybir.dt.float32

    # Bass's __init__ memsets four const buffers on gpsimd ahead of the
    # entry all-engine barrier; only const-float32-0.0 (the ACT bias
    # default) is ever read by this kernel, and the serial memsets delay
    # kernel start by ~0.3us. Skip the unused three.
    import concourse.bass as _bass

    _orig_memset = _bass.BassSharedVectorInterface.memset

    def _skip_unused_const_memset(self, ap, constant):
        name = getattr(ap.tensor, "name", "")
        if name.startswith("const-") and name != "const-float32-0.0":
            return None
        return _orig_memset(self, ap, constant)

    _bass.BassSharedVectorInterface.memset = _skip_unused_const_memset
    try:
        nc = bacc.Bacc(
            "TRN2", target_bir_lowering=False, debug=False, num_devices=N_CORES
        )
    finally:
        _bass.BassSharedVectorInterface.memset = _orig_memset
    c_in = nc.dram_tensor("c", [RB, N], f32, kind="ExternalInput").ap()
    # y holds only 0/1: shipped as int8 (value-exact) to quarter the DMA bytes
    y_in = nc.dram_tensor("y", [RB, N], mybir.dt.int8, kind="ExternalInput").ap()
    out = nc.dram_tensor("loss", [1, RB], f32, kind="ExternalOutput").ap()

    with tile.TileContext(nc) as tc:
        with tc.tile_pool(name="main", bufs=1) as pool:
            C = pool.tile([P, FD], f32)
            Y = pool.tile([P, FD], mybir.dt.int8)
            # spread DMAs over the three DGE-capable queues so the issues
            # overlap (only sync/scalar/gpsimd can initiate DMAs)
            # gpsimd's queue starts late (const-memset preamble), so keep
            # inputs on sync+scalar; the int8 y transfers are tiny
            nc.sync.dma_start(C[0:RB, :], c_in[:, 0:FD])
            nc.scalar.dma_start(C[RB:P, :], c_in[:, FD:N])
            nc.sync.dma_start(Y[0:RB, :], y_in[:, 0:FD])
            nc.scalar.dma_start(Y[RB:P, :], y_in[:, FD:N])

            # loss staging tile: col 0 gets per-row losses; memset first so
            # the 32x32 transpose reads no uninitialized columns
            LT = pool.tile([RB, 32], f32)
            nc.gpsimd.memset(LT[:], 0.0)

            # per-partition sums: S_v = sum(-2y); S_aw col0=SA, col1=sum((y-.5)A)
            S_v = pool.tile([P, 1], f32)
            S_aw = pool.tile([P, 2], f32)

            # g' = (y - 0.5) * c = -v*c/2   (critical path: feeds ACT chain)
            G = pool.tile([P, FD], f32)
            nc.vector.scalar_tensor_tensor(
                G[:], Y[:], 0.5, C[:], OP.subtract, OP.mult
            )
            # sum(-2y) per half-row via (y*-1)-y (out tile discarded);
            # tensor_scalar+accum rejects int input, STT+accum doesn't.
            VD = pool.tile([P, FD], f32)
            nc.vector.scalar_tensor_tensor(
                VD[:], Y[:], -1.0, Y[:], OP.mult, OP.subtract,
                accum_out=S_v[:],
            )

            # Denominator path depends only on y, so it runs on the DVE
            # hidden under the ACT tanh/exp passes.
            # Fold column halves: row sums = S[p] + S[p+64]. Both TT inputs
            # in SBUF must share a base partition (walrus NCC_IBIR297), so
            # first shift the upper half down with a copy.
            S2v = pool.tile([RB, 1], f32)
            nc.vector.tensor_copy(S2v[:], S_v[RB:P, :])
            F0 = pool.tile([RB, 1], f32)
            nc.vector.tensor_add(F0[:], S_v[0:RB, :], S2v[:])
            # den' = (F0 + 2N)*F0 = -(N^2 - SV^2)   [SV = N + F0]
            DEN = pool.tile([RB, 1], f32)
            nc.vector.scalar_tensor_tensor(
                DEN[:], F0[:], 2.0 * float(N), F0[:], OP.add, OP.mult
            )
            R = pool.tile([RB, 1], f32)
            nc.vector.reciprocal(R[:], DEN[:])

            # tanh(-g') = tanh(v*c/2)
            TH = pool.tile([P, FD], f32)
            nc.scalar.activation(TH[:], G[:], AF.Tanh, scale=-1.0)
            # A = exp(tanh(v*c/2)/2), row-half sums SA accumulated for free
            A = pool.tile([P, FD], f32)
            nc.scalar.activation(
                A[:], TH[:], AF.Exp, scale=0.5, accum_out=S_aw[:, 0:1]
            )

            # w = (y-0.5)*A = -v*A/2, accum -> -SW/2
            W = pool.tile([P, FD], f32)
            nc.vector.scalar_tensor_tensor(
                W[:], Y[:], 0.5, A[:], OP.subtract, OP.mult,
                accum_out=S_aw[:, 1:2],
            )

            # Late tail: fold SA/SW2 halves, then loss = num'/den'
            S2aw = pool.tile([RB, 2], f32)
            nc.vector.tensor_copy(S2aw[:], S_aw[RB:P, :])
            F12 = pool.tile([RB, 2], f32)
            nc.vector.tensor_add(F12[:], S_aw[0:RB, :], S2aw[:])
            P2 = pool.tile([RB, 2], f32)
            nc.vector.tensor_mul(P2[:], F12[:], F12[:])  # SA^2, SW^2/4
            # num' = 4*(SW^2/4) - SA^2 = -(SA^2 - SW^2)
            NUM = pool.tile([RB, 1], f32)
            nc.vector.scalar_tensor_tensor(
                NUM[:], P2[:, 1:2], 4.0, P2[:, 0:1], OP.mult, OP.subtract
            )
            # per-row loss into col 0 of the transpose staging tile
            nc.vector.tensor_mul(LT[:, 0:1], NUM[:], R[:])

            # Gather the [64,1] losses into single partitions: 32x32 block
            # transpose puts loss[0:32] on partition 0 and loss[32:64] on
            # partition 32; store each with its own single-descriptor DMA
            # on separate queues.
            TP = pool.tile([RB, 32], f32)
            nc.vector.transpose(TP[:], LT[:])

            nc.sync.dma_start(out[:, 0:32], TP[0:1, 0:32])
            nc.scalar.dma_start(out[:, 32:64], TP[32:33, 0:32])

    nc.compile()
    return nc


def get_nc():
    if "nc" not in _NC_CACHE:
        _NC_CACHE["nc"] = build_nc()
    return _NC_CACHE["nc"]


def make_in_maps(c, y):
    c = np.ascontiguousarray(np.asarray(c, dtype=np.float32))
    y = np.ascontiguousarray(np.asarray(y, dtype=np.int32))
    assert c.shape == (B, N) and y.shape == (B, N)
    return [
        {
            "c": np.ascontiguousarray(c[i * RB : (i + 1) * RB]),
            "y": np.ascontiguousarray(
                y[i * RB : (i + 1) * RB].astype(np.int8)
            ),
        }
        for i in range(N_CORES)
    ]


def kernel(c, y, _trace=False, _trace_kwargs=None):
    nc = get_nc()
    res = run_bass_kernel_spmd(
        nc,
        make_in_maps(c, y),
        list(range(N_CORES)),
        trace=_trace,
        **(_trace_kwargs or {}),
    )
    rows = np.concatenate(
        [res.results[i]["loss"][0, :] for i in range(N_CORES)]
    )
    loss = np.asarray(rows.astype(np.float64).mean(), dtype=np.float32)
    if _trace:
        return loss, res
    return loss


# revision 14
# speedup vs baseline: 1.0651x; 1.0651x over previous
"""BPMLL loss kernel for Trainium2, data-parallel over 8 NeuronCores.

Reference math (B=512 rows, n=512 labels, bias=(1,1)):
    s   = sigmoid(c)
    pos = sum_k y_k * exp(-s_k)         (per row)
    neg = sum_l (1-y_l) * exp(+s_l)     (per row)
    loss = mean( pos*neg / (|Y| * |Ybar|) )

Device formulation (avoids the sigmoid/exp table-set switch on the ACT
engine -- tanh and exp live in the same `exp_and_others` table set):
    v  = 1 - 2y  in {+1,-1}
    exp(v*s) = exp(v*(0.5 + 0.5*tanh(c/2))) = e^{v/2} * exp(0.5*tanh(v*c/2))
    A  = exp(0.5*tanh(v*c/2))            (tanh odd => tanh(v*c/2)=v*tanh(c/2))
    SA = sum_k A_k,  SW2 = sum_k (y_k-0.5)*A_k = -SW/2,  SVr = sum_k -2*y_k
    pos*neg       = (SA^2 - SW^2)/4      (the e^{+-1/2} factors cancel)
    |Y|*|Ybar|    = (n^2 - SV^2)/4,  SV = n + SVr
    loss_row      = (SA^2 - 4*SW2^2) / -((SVr + 2n)*SVr)

Each core processes 64 rows laid out as [128 partitions, 256 free]:
partition p<64 holds row p cols 0:256, partition p+64 holds row p cols
256:512. Row sums are recovered by folding partitions p and p+64.

The 64 per-row losses are gathered into one partition with a 32x32
stream transpose before the output DMA so the store is a single
contiguous 256B descriptor (a [64,1] per-partition store pays ~370ns
per DMA-semaphore increment, ~6us total).
"""

import numpy as np

import concourse.bacc as bacc
import concourse.mybir as mybir
import concourse.tile as tile
from concourse.bass_utils import run_bass_kernel_spmd

N_CORES = 8
B = 512          # batch rows
N = 512          # labels per row
RB = B // N_CORES  # 64 rows per core
H = 2              # column halves folded onto partitions
P = RB * H         # 128 partitions
FD = N // H        # 256 free elements per partition

_NC_CACHE = {}


def build_nc():
    AF = mybir.ActivationFunctionType
    OP = mybir.AluOpType
    f32 = mybir.dt.float32

    # Bass's __init__ memsets four const buffers on gpsimd ahead of the
    # entry all-engine barrier; only const-float32-0.0 (the ACT bias
    # default) is ever read by this kernel, and the serial memsets delay
    # kernel start by ~0.3us. Skip the unused three.
    import concourse.bass as _bass

    _cls = _bass.BassEitherVectorEngine
    _orig_memset = _cls.memset

    def _skip_unused_const_memset(self, ap, constant):
        name = getattr(ap.tensor, "name", "")
        if name.startswith("const-") and name != "const-float32-0.0":
            return None
        return _orig_memset(self, ap, constant)

    _cls.memset = _skip_unused_const_memset
    try:
        nc = bacc.Bacc(
            "TRN2", target_bir_lowering=False, debug=False, num_devices=N_CORES
        )
    finally:
        _cls.memset = _orig_memset
    c_in = nc.dram_tensor("c", [RB, N], f32, kind="ExternalInput").ap()
    # y holds only 0/1: shipped as int8 (value-exact) to quarter the DMA bytes
    y_in = nc.dram_tensor("y", [RB, N], mybir.dt.int8, kind="ExternalInput").ap()
    out = nc.dram_tensor("loss", [1, RB], f32, kind="ExternalOutput").ap()

    with tile.TileContext(nc) as tc:
        with tc.tile_pool(name="main", bufs=1) as pool:
            C = pool.tile([P, FD], f32)
            Y = pool.tile([P, FD], mybir.dt.int8)
            # spread DMAs over the three DGE-capable queues so the issues
            # overlap (only sync/scalar/gpsimd can initiate DMAs)
            # gpsimd's queue starts late (const-memset preamble), so keep
            # inputs on sync+scalar; the int8 y transfers are tiny
            nc.sync.dma_start(C[0:RB, :], c_in[:, 0:FD])
            nc.scalar.dma_start(C[RB:P, :], c_in[:, FD:N])
            nc.sync.dma_start(Y[0:RB, :], y_in[:, 0:FD])
            nc.scalar.dma_start(Y[RB:P, :], y_in[:, FD:N])

            # loss staging tile: col 0 gets per-row losses; memset first so
            # the 32x32 transpose reads no uninitialized columns
            LT = pool.tile([RB, 32], f32)
            nc.gpsimd.memset(LT[:], 0.0)

            # per-partition sums: S_v = sum(-2y); S_aw col0=SA, col1=sum((y-.5)A)
            S_v = pool.tile([P, 1], f32)
            S_aw = pool.tile([P, 2], f32)

            # g' = (y - 0.5) * c = -v*c/2   (critical path: feeds ACT chain)
            G = pool.tile([P, FD], f32)
            nc.vector.scalar_tensor_tensor(
                G[:], Y[:], 0.5, C[:], OP.subtract, OP.mult
            )
            # sum(-2y) per half-row via (y*-1)-y (out tile discarded);
            # tensor_scalar+accum rejects int input, STT+accum doesn't.
            VD = pool.tile([P, FD], f32)
            nc.vector.scalar_tensor_tensor(
                VD[:], Y[:], -1.0, Y[:], OP.mult, OP.subtract,
                accum_out=S_v[:],
            )

            # Denominator path depends only on y, so it runs on the DVE
            # hidden under the ACT tanh/exp passes.
            # Fold column halves: row sums = S[p] + S[p+64]. Both TT inputs
            # in SBUF must share a base partition (walrus NCC_IBIR297), so
            # first shift the upper half down with a copy.
            S2v = pool.tile([RB, 1], f32)
            nc.vector.tensor_copy(S2v[:], S_v[RB:P, :])
            F0 = pool.tile([RB, 1], f32)
            nc.vector.tensor_add(F0[:], S_v[0:RB, :], S2v[:])
            # den' = (F0 + 2N)*F0 = -(N^2 - SV^2)   [SV = N + F0]
            DEN = pool.tile([RB, 1], f32)
            nc.vector.scalar_tensor_tensor(
                DEN[:], F0[:], 2.0 * float(N), F0[:], OP.add, OP.mult
            )
            R = pool.tile([RB, 1], f32)
            nc.vector.reciprocal(R[:], DEN[:])

            # tanh(-g') = tanh(v*c/2)
            TH = pool.tile([P, FD], f32)
            nc.scalar.activation(TH[:], G[:], AF.Tanh, scale=-1.0)
            # A = exp(tanh(v*c/2)/2), row-half sums SA accumulated for free
            A = pool.tile([P, FD], f32)
            nc.scalar.activation(
                A[:], TH[:], AF.Exp, scale=0.5, accum_out=S_aw[:, 0:1]
            )

            # w = (y-0.5)*A = -v*A/2, accum -> -SW/2
            W = pool.tile([P, FD], f32)
            nc.vector.scalar_tensor_tensor(
                W[:], Y[:], 0.5, A[:], OP.subtract, OP.mult,
                accum_out=S_aw[:, 1:2],
            )

            # Late tail: fold SA/SW2 halves, then loss = num'/den'
            S2aw = pool.tile([RB, 2], f32)
            nc.vector.tensor_copy(S2aw[:], S_aw[RB:P, :])
            F12 = pool.tile([RB, 2], f32)
            nc.vector.tensor_add(F12[:], S_aw[0:RB, :], S2aw[:])
            P2 = pool.tile([RB, 2], f32)
            nc.vector.tensor_mul(P2[:], F12[:], F12[:])  # SA^2, SW^2/4
            # num' = 4*(SW^2/4) - SA^2 = -(SA^2 - SW^2)
            NUM = pool.tile([RB, 1], f32)
            nc.vector.scalar_tensor_tensor(
                NUM[:], P2[:, 1:2], 4.0, P2[:, 0:1], OP.mult, OP.subtract
            )
            # per-row loss into col 0 of the transpose staging tile
            nc.vector.tensor_mul(LT[:, 0:1], NUM[:], R[:])

            # Gather the [64,1] losses into one partition: 32x32 block
            # transpose puts loss[0:32] on partition 0 and loss[32:64] on
            # partition 32; one shifted copy concatenates them, then a
            # single-descriptor 256B DMA stores the row.
            TP = pool.tile([RB, 2 * 32], f32)
            nc.vector.transpose(TP[:, 0:32], LT[:])
            nc.vector.tensor_copy(TP[0:1, 32:64], TP[32:33, 0:32])

            nc.sync.dma_start(out, TP[0:1, 0:RB])

    nc.compile()
    return nc


def get_nc():
    if "nc" not in _NC_CACHE:
        _NC_CACHE["nc"] = build_nc()
    return _NC_CACHE["nc"]


def make_in_maps(c, y):
    c = np.ascontiguousarray(np.asarray(c, dtype=np.float32))
    y = np.ascontiguousarray(np.asarray(y, dtype=np.int32))
    assert c.shape == (B, N) and y.shape == (B, N)
    return [
        {
            "c": np.ascontiguousarray(c[i * RB : (i + 1) * RB]),
            "y": np.ascontiguousarray(
                y[i * RB : (i + 1) * RB].astype(np.int8)
            ),
        }
        for i in range(N_CORES)
    ]


def kernel(c, y, _trace=False, _trace_kwargs=None):
    nc = get_nc()
    res = run_bass_kernel_spmd(
        nc,
        make_in_maps(c, y),
        list(range(N_CORES)),
        trace=_trace,
        **(_trace_kwargs or {}),
    )
    rows = np.concatenate(
        [res.results[i]["loss"][0, :] for i in range(N_CORES)]
    )
    loss = np.asarray(rows.astype(np.float64).mean(), dtype=np.float32)
    if _trace:
        return loss, res
    return loss


# revision 15
# speedup vs baseline: 1.0934x; 1.0266x over previous
"""BPMLL loss kernel for Trainium2, data-parallel over 8 NeuronCores.

Reference math (B=512 rows, n=512 labels, bias=(1,1)):
    s   = sigmoid(c)
    pos = sum_k y_k * exp(-s_k)         (per row)
    neg = sum_l (1-y_l) * exp(+s_l)     (per row)
    loss = mean( pos*neg / (|Y| * |Ybar|) )

Device formulation (avoids the sigmoid/exp table-set switch on the ACT
engine -- tanh and exp live in the same `exp_and_others` table set):
    v  = 1 - 2y  in {+1,-1}
    exp(v*s) = exp(v*(0.5 + 0.5*tanh(c/2))) = e^{v/2} * exp(0.5*tanh(v*c/2))
    A  = exp(0.5*tanh(v*c/2))            (tanh odd => tanh(v*c/2)=v*tanh(c/2))
    SA = sum_k A_k,  SW2 = sum_k (y_k-0.5)*A_k = -SW/2,  SVr = sum_k -2*y_k
    pos*neg       = (SA^2 - SW^2)/4      (the e^{+-1/2} factors cancel)
    |Y|*|Ybar|    = (n^2 - SV^2)/4,  SV = n + SVr
    loss_row      = (SA^2 - 4*SW2^2) / -((SVr + 2n)*SVr)

Each core processes 64 rows laid out as [128 partitions, 256 free]:
partition p<64 holds row p cols 0:256, partition p+64 holds row p cols
256:512. Row sums are recovered by folding partitions p and p+64.

The 64 per-row losses are gathered into one partition with a 32x32
stream transpose before the output DMA so the store is a single
contiguous 256B descriptor (a [64,1] per-partition store pays ~370ns
per DMA-semaphore increment, ~6us total).
"""

import numpy as np

import concourse.bacc as bacc
import concourse.mybir as mybir
import concourse.tile as tile
from concourse.bass_utils import run_bass_kernel_spmd

N_CORES = 8
B = 512          # batch rows
N = 512          # labels per row
RB = B // N_CORES  # 64 rows per core
H = 2              # column halves folded onto partitions
P = RB * H         # 128 partitions
FD = N // H        # 256 free elements per partition

_NC_CACHE = {}


def build_nc():
    AF = mybir.ActivationFunctionType
    OP = mybir.AluOpType
    f32 = mybir.dt.float32

    # Bass's __init__ memsets four const buffers on gpsimd ahead of the
    # entry all-engine barrier; only const-float32-0.0 (the ACT bias
    # default) is ever read by this kernel, and the serial memsets delay
    # kernel start by ~0.3us. Skip the unused three (best-effort; fall
    # back to vanilla construction if bass internals moved).
    def _make_nc():
        return bacc.Bacc(
            "TRN2", target_bir_lowering=False, debug=False, num_devices=N_CORES
        )

    try:
        import concourse.bass as _bass

        _cls = _bass.BassEitherVectorEngine
        _orig_memset = _cls.memset

        def _skip_unused_const_memset(self, ap, constant):
            name = getattr(getattr(ap, "tensor", None), "name", "")
            if name.startswith("const-") and name != "const-float32-0.0":
                return None
            return _orig_memset(self, ap, constant)

        _cls.memset = _skip_unused_const_memset
        try:
            nc = _make_nc()
        finally:
            _cls.memset = _orig_memset
    except AttributeError:
        nc = _make_nc()
    c_in = nc.dram_tensor("c", [RB, N], f32, kind="ExternalInput").ap()
    # y holds only 0/1: shipped as int8 (value-exact) to quarter the DMA bytes
    y_in = nc.dram_tensor("y", [RB, N], mybir.dt.int8, kind="ExternalInput").ap()
    out = nc.dram_tensor("loss", [1, RB], f32, kind="ExternalOutput").ap()

    with tile.TileContext(nc) as tc:
        with tc.tile_pool(name="main", bufs=1) as pool:
            C = pool.tile([P, FD], f32)
            Y = pool.tile([P, FD], mybir.dt.int8)
            # spread DMAs over the three DGE-capable queues so the issues
            # overlap (only sync/scalar/gpsimd can initiate DMAs)
            # gpsimd's queue starts late (const-memset preamble), so keep
            # inputs on sync+scalar; the int8 y transfers are tiny
            nc.sync.dma_start(C[0:RB, :], c_in[:, 0:FD])
            nc.scalar.dma_start(C[RB:P, :], c_in[:, FD:N])
            nc.sync.dma_start(Y[0:RB, :], y_in[:, 0:FD])
            nc.scalar.dma_start(Y[RB:P, :], y_in[:, FD:N])

            # loss staging tile: col 0 gets per-row losses; memset first so
            # the 32x32 transpose reads no uninitialized columns
            LT = pool.tile([RB, 32], f32)
            nc.gpsimd.memset(LT[:], 0.0)

            # per-partition sums: S_v = sum(-2y); S_aw col0=SA, col1=sum((y-.5)A)
            S_v = pool.tile([P, 1], f32)
            S_aw = pool.tile([P, 2], f32)

            # g' = (y - 0.5) * c = -v*c/2   (critical path: feeds ACT chain)
            G = pool.tile([P, FD], f32)
            nc.vector.scalar_tensor_tensor(
                G[:], Y[:], 0.5, C[:], OP.subtract, OP.mult
            )
            # sum(-2y) per half-row via (y*-1)-y (out tile discarded);
            # tensor_scalar+accum rejects int input, STT+accum doesn't.
            VD = pool.tile([P, FD], f32)
            nc.vector.scalar_tensor_tensor(
                VD[:], Y[:], -1.0, Y[:], OP.mult, OP.subtract,
                accum_out=S_v[:],
            )

            # Denominator path depends only on y, so it runs on the DVE
            # hidden under the ACT tanh/exp passes.
            # Fold column halves: row sums = S[p] + S[p+64]. Both TT inputs
            # in SBUF must share a base partition (walrus NCC_IBIR297), so
            # first shift the upper half down with a copy.
            S2v = pool.tile([RB, 1], f32)
            nc.vector.tensor_copy(S2v[:], S_v[RB:P, :])
            F0 = pool.tile([RB, 1], f32)
            nc.vector.tensor_add(F0[:], S_v[0:RB, :], S2v[:])
            # den' = (F0 + 2N)*F0 = -(N^2 - SV^2)   [SV = N + F0]
            DEN = pool.tile([RB, 1], f32)
            nc.vector.scalar_tensor_tensor(
                DEN[:], F0[:], 2.0 * float(N), F0[:], OP.add, OP.mult
            )
            R = pool.tile([RB, 1], f32)
            nc.vector.reciprocal(R[:], DEN[:])

            # tanh(-g') = tanh(v*c/2)
            TH = pool.tile([P, FD], f32)
            nc.scalar.activation(TH[:], G[:], AF.Tanh, scale=-1.0)
            # A = exp(tanh(v*c/2)/2), row-half sums SA accumulated for free
            A = pool.tile([P, FD], f32)
            nc.scalar.activation(
                A[:], TH[:], AF.Exp, scale=0.5, accum_out=S_aw[:, 0:1]
            )

            # w = (y-0.5)*A = -v*A/2, accum -> -SW/2
            W = pool.tile([P, FD], f32)
            nc.vector.scalar_tensor_tensor(
                W[:], Y[:], 0.5, A[:], OP.subtract, OP.mult,
                accum_out=S_aw[:, 1:2],
            )

            # Late tail: fold SA/SW2 halves, then loss = num'/den'
            S2aw = pool.tile([RB, 2], f32)
            nc.vector.tensor_copy(S2aw[:], S_aw[RB:P, :])
            F12 = pool.tile([RB, 2], f32)
            nc.vector.tensor_add(F12[:], S_aw[0:RB, :], S2aw[:])
            P2 = pool.tile([RB, 2], f32)
            nc.vector.tensor_mul(P2[:], F12[:], F12[:])  # SA^2, SW^2/4
            # num' = 4*(SW^2/4) - SA^2 = -(SA^2 - SW^2)
            NUM = pool.tile([RB, 1], f32)
            nc.vector.scalar_tensor_tensor(
                NUM[:], P2[:, 1:2], 4.0, P2[:, 0:1], OP.mult, OP.subtract
            )
            # per-row loss into col 0 of the transpose staging tile
            nc.vector.tensor_mul(LT[:, 0:1], NUM[:], R[:])

            # Gather the [64,1] losses into one partition: 32x32 block
            # transpose puts loss[0:32] on partition 0 and loss[32:64] on
            # partition 32; one shifted copy concatenates them, then a
            # single-descriptor 256B DMA stores the row.
            TP = pool.tile([RB, 2 * 32], f32)
            nc.vector.transpose(TP[:, 0:32], LT[:])
            nc.vector.tensor_copy(TP[0:1, 32:64], TP[32:33, 0:32])

            nc.sync.dma_start(out, TP[0:1, 0:RB])

    nc.compile()
    return nc


def get_nc():
    if "nc" not in _NC_CACHE:
        _NC_CACHE["nc"] = build_nc()
    return _NC_CACHE["nc"]


def make_in_maps(c, y):
    c = np.ascontiguousarray(np.asarray(c, dtype=np.float32))
    y = np.ascontiguousarray(np.asarray(y, dtype=np.int32))
    assert c.shape == (B, N) and y.shape == (B, N)
    return [
        {
            "c": np.ascontiguousarray(c[i * RB : (i + 1) * RB]),
            "y": np.ascontiguousarray(
                y[i * RB : (i + 1) * RB].astype(np.int8)
            ),
        }
        for i in range(N_CORES)
    ]


def kernel(c, y, _trace=False, _trace_kwargs=None):
    nc = get_nc()
    res = run_bass_kernel_spmd(
        nc,
        make_in_maps(c, y),
        list(range(N_CORES)),
        trace=_trace,
        **(_trace_kwargs or {}),
    )
    rows = np.concatenate(
        [res.results[i]["loss"][0, :] for i in range(N_CORES)]
    )
    loss = np.asarray(rows.astype(np.float64).mean(), dtype=np.float32)
    if _trace:
        return loss, res
    return loss


# revision 17
# speedup vs baseline: 1.1195x; 1.0238x over previous
"""BPMLL loss kernel for Trainium2, data-parallel over 8 NeuronCores.

Reference math (B=512 rows, n=512 labels, bias=(1,1)):
    s   = sigmoid(c)
    pos = sum_k y_k * exp(-s_k)         (per row)
    neg = sum_l (1-y_l) * exp(+s_l)     (per row)
    loss = mean( pos*neg / (|Y| * |Ybar|) )

Device formulation (avoids the sigmoid/exp table-set switch on the ACT
engine -- tanh and exp live in the same `exp_and_others` table set):
    v  = 1 - 2y  in {+1,-1}
    exp(v*s) = exp(v*(0.5 + 0.5*tanh(c/2))) = e^{v/2} * exp(0.5*tanh(v*c/2))
    A  = exp(0.5*tanh(v*c/2))            (tanh odd => tanh(v*c/2)=v*tanh(c/2))
    SA = sum_k A_k,  SW2 = sum_k (y_k-0.5)*A_k = -SW/2,  SVr = sum_k -2*y_k
    pos*neg       = (SA^2 - SW^2)/4      (the e^{+-1/2} factors cancel)
    |Y|*|Ybar|    = (n^2 - SV^2)/4,  SV = n + SVr
    loss_row      = (SA^2 - 4*SW2^2) / -((SVr + 2n)*SVr)

Each core processes 64 rows laid out as [128 partitions, 256 free]:
partition p<64 holds row p cols 0:256, partition p+64 holds row p cols
256:512. Row sums are recovered by folding partitions p and p+64.

The 64 per-row losses are gathered into one partition with a 32x32
stream transpose before the output DMA so the store is a single
contiguous 256B descriptor (a [64,1] per-partition store pays ~370ns
per DMA-semaphore increment, ~6us total).
"""

import numpy as np

import concourse.bacc as bacc
import concourse.mybir as mybir
import concourse.tile as tile
from concourse.bass_utils import run_bass_kernel_spmd

N_CORES = 8
B = 512          # batch rows
N = 512          # labels per row
RB = B // N_CORES  # 64 rows per core
H = 2              # column halves folded onto partitions
P = RB * H         # 128 partitions
FD = N // H        # 256 free elements per partition

_NC_CACHE = {}


def build_nc():
    AF = mybir.ActivationFunctionType
    OP = mybir.AluOpType
    f32 = mybir.dt.float32

    # Bass's __init__ memsets four const buffers on gpsimd ahead of the
    # entry all-engine barrier; only const-float32-0.0 (the ACT bias
    # default) is ever read by this kernel, and the serial memsets delay
    # kernel start by ~0.3us. Skip the unused three (best-effort; fall
    # back to vanilla construction if bass internals moved).
    def _make_nc():
        return bacc.Bacc(
            "TRN2", target_bir_lowering=False, debug=False, num_devices=N_CORES
        )

    try:
        import concourse.bass as _bass

        _cls = _bass.BassEitherVectorEngine
        _orig_memset = _cls.memset

        def _skip_unused_const_memset(self, ap, constant):
            name = getattr(getattr(ap, "tensor", None), "name", "")
            if name.startswith("const-") and name != "const-float32-0.0":
                return None
            return _orig_memset(self, ap, constant)

        _cls.memset = _skip_unused_const_memset
        try:
            nc = _make_nc()
        finally:
            _cls.memset = _orig_memset
    except AttributeError:
        nc = _make_nc()
    # Host packs c (f32) and y (0/1, value-exact as int8 bytes) per core
    # into one [128, 320] f32 buffer already in the folded partition
    # layout, so each DGE queue does exactly one first-in-queue DMA
    # (a queue's second dma_start serializes ~0.65us behind the first).
    YF = FD // 4  # y bytes viewed as f32 columns
    pk_in = nc.dram_tensor("pk", [P, FD + YF], f32, kind="ExternalInput").ap()
    out = nc.dram_tensor("loss", [1, RB], f32, kind="ExternalOutput").ap()

    with tile.TileContext(nc) as tc:
        with tc.tile_pool(name="main", bufs=1) as pool:
            PK = pool.tile([P, FD + YF], f32)
            nc.sync.dma_start(PK[0:RB, :], pk_in[0:RB, :])
            nc.scalar.dma_start(PK[RB:P, :], pk_in[RB:P, :])
            C = PK[:, 0:FD]
            Y = PK[:].bitcast(mybir.dt.int8)[:, 4 * FD : 4 * FD + FD]

            # loss staging tile: col 0 gets per-row losses; memset first so
            # the 32x32 transpose reads no uninitialized columns
            LT = pool.tile([RB, 32], f32)
            nc.gpsimd.memset(LT[:], 0.0)

            # per-partition sums: S_v = sum(-2y); S_aw col0=SA, col1=sum((y-.5)A)
            S_v = pool.tile([P, 1], f32)
            S_aw = pool.tile([P, 2], f32)

            # g' = (y - 0.5) * c = -v*c/2   (critical path: feeds ACT chain)
            G = pool.tile([P, FD], f32)
            nc.vector.scalar_tensor_tensor(
                G[:], Y, 0.5, C, OP.subtract, OP.mult
            )
            # sum(-2y) per half-row via (y*-1)-y (out tile discarded);
            # tensor_scalar+accum rejects int input, STT+accum doesn't.
            VD = pool.tile([P, FD], f32)
            nc.vector.scalar_tensor_tensor(
                VD[:], Y, -1.0, Y, OP.mult, OP.subtract,
                accum_out=S_v[:],
            )

            # Denominator path depends only on y, so it runs on the DVE
            # hidden under the ACT tanh/exp passes.
            # Fold column halves: row sums = S[p] + S[p+64]. Both TT inputs
            # in SBUF must share a base partition (walrus NCC_IBIR297), so
            # first shift the upper half down with a copy.
            S2v = pool.tile([RB, 1], f32)
            nc.vector.tensor_copy(S2v[:], S_v[RB:P, :])
            F0 = pool.tile([RB, 1], f32)
            nc.vector.tensor_add(F0[:], S_v[0:RB, :], S2v[:])
            # den' = (F0 + 2N)*F0 = -(N^2 - SV^2)   [SV = N + F0]
            DEN = pool.tile([RB, 1], f32)
            nc.vector.scalar_tensor_tensor(
                DEN[:], F0[:], 2.0 * float(N), F0[:], OP.add, OP.mult
            )
            R = pool.tile([RB, 1], f32)
            nc.vector.reciprocal(R[:], DEN[:])

            # tanh(-g') = tanh(v*c/2)
            TH = pool.tile([P, FD], f32)
            nc.scalar.activation(TH[:], G[:], AF.Tanh, scale=-1.0)
            # A = exp(tanh(v*c/2)/2), row-half sums SA accumulated for free
            A = pool.tile([P, FD], f32)
            nc.scalar.activation(
                A[:], TH[:], AF.Exp, scale=0.5, accum_out=S_aw[:, 0:1]
            )

            # w = (y-0.5)*A = -v*A/2, accum -> -SW/2
            W = pool.tile([P, FD], f32)
            nc.vector.scalar_tensor_tensor(
                W[:], Y, 0.5, A[:], OP.subtract, OP.mult,
                accum_out=S_aw[:, 1:2],
            )

            # Late tail: fold SA/SW2 halves, then loss = num'/den'
            S2aw = pool.tile([RB, 2], f32)
            nc.vector.tensor_copy(S2aw[:], S_aw[RB:P, :])
            F12 = pool.tile([RB, 2], f32)
            nc.vector.tensor_add(F12[:], S_aw[0:RB, :], S2aw[:])
            P2 = pool.tile([RB, 2], f32)
            nc.vector.tensor_mul(P2[:], F12[:], F12[:])  # SA^2, SW^2/4
            # num' = 4*(SW^2/4) - SA^2 = -(SA^2 - SW^2)
            NUM = pool.tile([RB, 1], f32)
            nc.vector.scalar_tensor_tensor(
                NUM[:], P2[:, 1:2], 4.0, P2[:, 0:1], OP.mult, OP.subtract
            )
            # per-row loss into col 0 of the transpose staging tile
            nc.vector.tensor_mul(LT[:, 0:1], NUM[:], R[:])

            # Gather the [64,1] losses: 32x32 block transpose puts
            # loss[0:32] on partition 0 and loss[32:64] on partition 32;
            # one partition-strided two-descriptor DMA stores both rows.
            TP = pool.tile([RB, 32], f32)
            nc.vector.transpose(TP[:], LT[:])

            nc.sync.dma_start(
                out.rearrange("o (p f) -> (o p) f", p=2), TP[0:RB:32, :]
            )

    nc.compile()
    return nc


def get_nc():
    if "nc" not in _NC_CACHE:
        _NC_CACHE["nc"] = build_nc()
    return _NC_CACHE["nc"]


def make_in_maps(c, y):
    c = np.ascontiguousarray(np.asarray(c, dtype=np.float32))
    y = np.ascontiguousarray(np.asarray(y, dtype=np.int32))
    assert c.shape == (B, N) and y.shape == (B, N)
    y8 = y.astype(np.int8)
    maps = []
    for i in range(N_CORES):
        ci = c[i * RB : (i + 1) * RB]
        yi = y8[i * RB : (i + 1) * RB]
        pk = np.empty((P, FD + FD // 4), dtype=np.float32)
        for h in range(H):
            rows = slice(h * RB, (h + 1) * RB)
            cols = slice(h * FD, (h + 1) * FD)
            pk[rows, 0:FD] = ci[:, cols]
            pk[rows, FD:] = np.ascontiguousarray(yi[:, cols]).view(np.float32)
        maps.append({"pk": pk})
    return maps


def kernel(c, y, _trace=False, _trace_kwargs=None):
    nc = get_nc()
    res = run_bass_kernel_spmd(
        nc,
        make_in_maps(c, y),
        list(range(N_CORES)),
        trace=_trace,
        **(_trace_kwargs or {}),
    )
    rows = np.concatenate(
        [res.results[i]["loss"][0, :] for i in range(N_CORES)]
    )
    loss = np.asarray(rows.astype(np.float64).mean(), dtype=np.float32)
    if _trace:
        return loss, res
    return loss
